# revision 5
# baseline (speedup 1.0000x reference)
"""MaxMarginCriterion loss on 8 TRN2 NeuronCores (Bass/Tile).

reference:
    correct_sim[r] = cossim[r, argmax(target[r])]
    loss = mean_r( sum_c( relu(MARGIN + cossim - correct_sim) * (1 - target) ) )

Identity used on-device (target is exactly one-hot, so cossim[r, correct] ==
correct_sim[r] exactly and the correct column contributes relu(MARGIN) ==
MARGIN to the unmasked sum):
    row_sum[r] = sum_c relu(MARGIN + cossim[r, c] - correct_sim[r])
    loss = (sum_r row_sum[r] - MARGIN * N) / N

Sharding: data-parallel over the batch axis — core k handles rows
[k*2048, (k+1)*2048). Each core computes per-partition partial sums
(output [128, 16]); the final reduction over 8*128*16 floats happens on
host (the "all-reduce mean" of the sharding hint).

Per 128-row tile on device:
    DMA  cossim tile  [128, 2048] f32   (1 MiB, qSPDynamicHW ring)
    DMA  target tile  [128, 4096] i32   (2 MiB; int64 viewed as int32
         pairs, little-endian) — split into two 1 MiB halves, one on the
         SP HWDGE ring and one on the ACT HWDGE ring. The kernel is
         purely DMA-ingest-bound (~450 GB/s/core best observed, ≈ the
         16-SDMA-engine limit); feeding both descriptor rings keeps the
         engines busy across DMA handoffs — measured 5-20% faster than
         a single ring, never slower. SWDGE (gpsimd) and 4-way splits
         measured strictly worse; DMA-cast (int32->bf16) saves SBUF
         writes but not ingest, no gain.
    DVE  scalar_tensor_tensor: prod = cos * t_low(int32, stride-2, HW-cast),
         accum_out -> corr = sum(prod)
    DVE  tensor_scalar: bias = MARGIN - corr
    ACT  activation Relu(cos + bias), accum_out -> acc[:, i]
Compute overlaps DMA fully (DVE ~2.3 us + ACT ~2 us vs ~7 us DMA/tile).

(tensor_tensor_reduce is avoided: its TENSOR_TENSOR_REDUCE opcode wedges the
exec unit on this runtime; InstTensorScalarPtr/scalar_tensor_tensor with
accum_out does the same fused multiply+row-sum and runs fine.)
"""

import time

import numpy as np

import concourse.bacc as bacc
import concourse.tile as tile
from concourse import mybir
from concourse.bass_utils import run_bass_kernel_spmd

MARGIN = 0.1
N, C = 16384, 2048
NCORES = 8
ROWS = N // NCORES        # rows per core
P = 128                   # SBUF partitions
NT = ROWS // P            # 128-row tiles per core

_NC_CACHE = {}


def _build(reps=1):
    nc = bacc.Bacc("TRN2", target_bir_lowering=False, debug=False)
    cos = nc.dram_tensor("cossim", [ROWS, C], mybir.dt.float32, kind="ExternalInput").ap()
    tgt = nc.dram_tensor("target32", [ROWS, 2 * C], mybir.dt.int32, kind="ExternalInput").ap()
    out = nc.dram_tensor("out", [P, NT], mybir.dt.float32, kind="ExternalOutput").ap()

    with tile.TileContext(nc) as tc:
        with (
            tc.tile_pool(name="io", bufs=3) as io_pool,
            tc.tile_pool(name="work", bufs=3) as work,
            tc.tile_pool(name="accp", bufs=1) as accp,
        ):
            acc = accp.tile([P, NT], mybir.dt.float32)
            for r in range(reps):
                for i in range(NT):
                    cos_t = io_pool.tile([P, C], mybir.dt.float32, tag="cos")
                    tgt_t = io_pool.tile([P, 2 * C], mybir.dt.int32, tag="tgt")
                    nc.sync.dma_start(out=cos_t, in_=cos[i * P:(i + 1) * P, :])
                    # tgt split across the two HWDGE rings (SP + ACT): keeps
                    # both descriptor queues fed so the 16 SDMA engines never
                    # starve at DMA handoffs — measured ~5-20% faster than a
                    # single qSPDynamicHW ring, never slower. The split is
                    # uneven (0.5 MiB to SP, 1.5 MiB to ACT) so each ring
                    # carries 1.5 MiB/tile total including the 1 MiB cos on
                    # SP; byte-balanced rings beat the even 2/1 split on all
                    # estimators (and ACT carrying the majority loses — its
                    # sequencer also runs the activations).
                    cut = (2 * C) // 4
                    tsrc = tgt[i * P:(i + 1) * P, :]
                    nc.sync.dma_start(out=tgt_t[:, :cut], in_=tsrc[:, :cut])
                    nc.scalar.dma_start(out=tgt_t[:, cut:], in_=tsrc[:, cut:])
                    # low 32-bit words of the int64 one-hot: stride-2 view
                    t_low = tgt_t.rearrange("p (c two) -> p c two", two=2)[:, :, 0]

                    prod = work.tile([P, C], mybir.dt.float32, tag="prod")
                    corr = work.tile([P, 1], mybir.dt.float32, tag="corr")
                    nc.vector.scalar_tensor_tensor(
                        out=prod, in0=cos_t, scalar=1.0, in1=t_low,
                        op0=mybir.AluOpType.mult, op1=mybir.AluOpType.mult,
                        accum_out=corr,
                    )
                    bias = work.tile([P, 1], mybir.dt.float32, tag="bias")
                    nc.vector.tensor_scalar(
                        out=bias, in0=corr, scalar1=-1.0, scalar2=MARGIN,
                        op0=mybir.AluOpType.mult, op1=mybir.AluOpType.add,
                    )
                    relu = work.tile([P, C], mybir.dt.float32, tag="relu")
                    nc.scalar.activation(
                        out=relu, in_=cos_t,
                        func=mybir.ActivationFunctionType.Relu,
                        bias=bias, scale=1.0,
                        accum_out=acc[:, i:i + 1],
                    )
            nc.sync.dma_start(out=out, in_=acc)
    nc.compile()
    return nc


def _get_nc():
    if "nc" not in _NC_CACHE:
        _NC_CACHE["nc"] = _build()
    return _NC_CACHE["nc"]


def _run(cossim, target, trace=False, trace_kwargs=None):
    cossim = np.ascontiguousarray(np.asarray(cossim), dtype=np.float32)
    t = np.asarray(target)
    if t.dtype != np.int64:
        t = t.astype(np.int64)
    t32 = np.ascontiguousarray(t).view(np.int32).reshape(N, 2 * C)

    nc = _get_nc()
    in_maps = [
        {
            "cossim": cossim[k * ROWS:(k + 1) * ROWS],
            "target32": t32[k * ROWS:(k + 1) * ROWS],
        }
        for k in range(NCORES)
    ]
    # The shared device occasionally starts wedged from a prior tenant
    # (NRT_EXEC_UNIT_UNRECOVERABLE / "mesh desynced") and recovers within
    # ~a minute; retry rather than fail the whole call. Non-transient
    # errors (bad imports, shape/type bugs) re-raise immediately.
    for attempt in range(3):
        try:
            res = run_bass_kernel_spmd(
                nc, in_maps, core_ids=list(range(NCORES)),
                trace=trace, **(trace_kwargs or {}),
            )
            break
        except (ImportError, AssertionError, TypeError, ValueError, KeyError):
            raise
        except Exception:  # jax.errors.JaxRuntimeError et al.
            if attempt == 2:
                raise
            time.sleep(60)
    total = 0.0
    for k in range(NCORES):
        total += res.results[k]["out"].sum(dtype=np.float64)
    loss = (total - MARGIN * N) / N
    return np.asarray(loss, dtype=np.float32), res


def kernel(cossim, target):
    loss, _ = _run(cossim, target)
    return loss

